# revision 10
# baseline (speedup 1.0000x reference)
"""Trainium2 Bass kernel for nn_Attention_12695923327433 (8-core SPMD).

Sharding: batch(4) x H-strips(2) -> 8 cores. Each core computes a
(384, 64, 128) slice of the output. Cross-core comm: one tiny AllReduce of
per-head gram matrices (for the l2norm + q@k attention logits) between the
two strip-cores of each batch sample.

v2 schedule. The depthwise 3x3 convs dominate; each 512-col chunk is
assigned to one of three engine classes to balance occupancy:
  'pe'  9 PSUM-accumulated diagonal matmuls + ACT cast,
  'dve' 9 tensor_scalar muls + 8 tensor_tensor adds in bf16,
  'mix' 9 ACT per-partition muls + 8 DVE adds.
Other deltas vs v1: x streams in kk-interleaved column blocks so the
first matmul starts early; gram transposes are split across the sync
and scalar HWDGE queues; softmax + M-build run right after the
AllReduce inside phase B so the out stage never waits on attention
weights; pos-dw1 tiles are interleaved with the fused out chunks.
"""
import sys
sys.path.insert(0, "/opt/trn_rl_repo")
import numpy as np
import ml_dtypes

BF = ml_dtypes.bfloat16
DIM, HEADS, NCORES = 384, 8, 8
ROWS, COLS = 70, 130          # 3+64+3 rows, 1+128+1 cols (zero-padded halo)
NPOS = ROWS * COLS            # 9100
PAD = 132                     # flat guard so shifted APs stay in-bounds
FLAT = PAD + NPOS + PAD
CH = 512

_CACHE = {}

# engine class per dw chunk index (tuned for per-phase engine balance)
QK_PAT = ['pe', 'dve', 'pe', 'dve', 'pe', 'dve', 'pe', 'mix', 'pe',
          'dve', 'pe', 'dve', 'pe', 'dve', 'pe', 'pe', 'pe']       # 17
V_PAT = ['pe', 'dve', 'pe', 'dve', 'pe', 'dve', 'pe', 'dve', 'pe',
         'dve', 'pe', 'dve', 'pe', 'dve', 'pe', 'pe', 'pe', 'pe']  # 18
DW1_PAT = ['dve', 'mix', 'dve', 'pe', 'dve', 'mix', 'dve', 'pe', 'dve',
           'mix', 'dve', 'pe', 'dve', 'dve', 'pe', 'pe', 'dve']    # 17
TR_SCALAR_MOD = 6             # every 6th gram transpose issues on nc.scalar


def _build(wsm, temp):
    from concourse import bass, bacc, tile, mybir

    f32 = mybir.dt.float32
    bf16 = mybir.dt.bfloat16
    MM = mybir.AluOpType.mult
    ADD = mybir.AluOpType.add
    AX = mybir.AxisListType.X
    ACT = mybir.ActivationFunctionType

    nc = bacc.Bacc("TRN2", target_bir_lowering=False, debug=False,
                   num_devices=NCORES)

    x_d = nc.dram_tensor("x", [DIM, NPOS], bf16, kind="ExternalInput")
    wT_d = nc.dram_tensor("wT", [DIM, 3 * DIM], bf16, kind="ExternalInput")
    dwd_d = nc.dram_tensor("dwd", [15, 9, 128, 128], bf16, kind="ExternalInput")
    projT_d = nc.dram_tensor("projT", [HEADS, 48, DIM], bf16, kind="ExternalInput")
    mm_d = nc.dram_tensor("maskmul", [48, HEADS * 48], f32, kind="ExternalInput")
    id_d = nc.dram_tensor("ident", [128, 128], bf16, kind="ExternalInput")
    id48_d = nc.dram_tensor("ident48", [48, 48], f32, kind="ExternalInput")
    ez_d = nc.dram_tensor("ez", [128, 2], f32, kind="ExternalInput")
    dwc_d = nc.dram_tensor("dwcol", [15, 9, 128], f32, kind="ExternalInput")
    out_d = nc.dram_tensor("out", [DIM, 64 * 128], f32, kind="ExternalOutput")

    # flat-region chunking (rows of the 70x130 halo grid, 512-wide chunks)
    full_chunks = [(s, min(NPOS, s + CH)) for s in range(0, NPOS, CH)]
    qk_u_chunks = [(s, min(68 * COLS, s + CH))
                   for s in range(2 * COLS, 68 * COLS, CH)]
    qk_dw_chunks = [(s, min(67 * COLS, s + CH))
                    for s in range(3 * COLS, 67 * COLS, CH)]
    v_dw_chunks = [(s, min(69 * COLS, s + CH))
                   for s in range(1 * COLS, 69 * COLS, CH)]
    dw1_chunks = [(s, min(68 * COLS, s + CH))
                  for s in range(2 * COLS, 68 * COLS, CH)]

    # payload block for head h (even heads on psum partitions 0-47,
    # odd heads on 64-111; payload blocks even-first)
    def blk(h):
        return (h % 2) * 4 + h // 2

    with tile.TileContext(nc) as tc:
        with tc.tile_pool(name="const", bufs=1) as cp, \
             tc.tile_pool(name="persist", bufs=1) as pp, \
             tc.tile_pool(name="dramp", bufs=1, space="DRAM") as dp:

            # ---- constants (before x so compute starts early) ----
            wT_sb = []
            for kk in range(3):
                t = cp.tile([128, 3 * DIM], bf16, tag=f"wT{kk}", name=f"wT{kk}")
                nc.sync.dma_start(t, wT_d.ap()[128 * kk:128 * kk + 128, :])
                wT_sb.append(t)
            dwcT = []
            for t9 in range(15):
                t = cp.tile([128, 9], f32, tag=f"dwc{t9}", name=f"dwc{t9}")
                nc.sync.dma_start(t, dwc_d.ap()[t9].transpose([1, 0]))
                dwcT.append(t)
            ez_sb = cp.tile([128, 2], f32, tag="ez", name="ez")
            nc.sync.dma_start(ez_sb, ez_d.ap())

            G_sb = pp.tile([48, 2048], f32, tag="gsb", name="G_sb")
            attn_bf = [pp.tile([48, 48], bf16, tag=f"at{h}", name=f"atl{h}")
                       for h in range(HEADS)]
            MT_sb = [pp.tile([128, DIM], bf16, tag=f"mt{i}", name=f"mtl{i}")
                     for i in range(3)]
            v_sb = [None] * 3

            qk_dram = dp.tile([6, 128, 64 * 128], bf16, tag="qkspill",
                              name="qk_dram")
            n2_dram = dp.tile([6, 128], f32, tag="n2", name="n2")
            cc_in = dp.tile([48, 2048], f32, tag="ccin", name="ccin")
            cc_out = dp.tile([48, 2048], f32, tag="ccout", name="ccout")

            # ================= Phase B + gram =================
            with tc.tile_pool(name="pb", bufs=1) as pb, \
                 tc.tile_pool(name="psB", bufs=1, space="PSUM") as psB:
                x_sb = [pb.tile([128, NPOS], bf16, tag=f"x{kk}", name=f"x{kk}")
                        for kk in range(3)]
                # tile-0 taps first, then x in kk-interleaved column blocks
                dwd_t0 = []
                for tap in range(9):
                    dt_ = pb.tile([128, 128], bf16, tag="dwd", bufs=12,
                                  name=f"dwd0_{tap}")
                    nc.sync.dma_start(dt_, dwd_d.ap()[0, tap])
                    dwd_t0.append(dt_)
                NB = 6
                for b6 in range(NB):
                    s = (NPOS // NB) * b6
                    e = NPOS if b6 == NB - 1 else (NPOS // NB) * (b6 + 1)
                    for kk in range(3):
                        nc.sync.dma_start(x_sb[kk][:, s:e],
                                          x_d.ap()[128 * kk:128 * kk + 128, s:e])
                    if b6 == 0:
                        # late-consumed constants ride behind the first block
                        projT_sb = []
                        for h in range(HEADS):
                            t = cp.tile([48, DIM], bf16, tag=f"pjT{h}",
                                        name=f"pjT{h}")
                            nc.sync.dma_start(t, projT_d.ap()[h])
                            projT_sb.append(t)
                        ident_sb = cp.tile([128, 128], bf16, tag="ident",
                                           name="ident")
                        nc.sync.dma_start(ident_sb, id_d.ap())
                        id48_sb = cp.tile([48, 48], f32, tag="id48",
                                          name="id48")
                        nc.sync.dma_start(id48_sb, id48_d.ap())
                        mm_sb = cp.tile([48, HEADS * 48], f32, tag="mm",
                                        name="mm")
                        nc.sync.dma_start(mm_sb, mm_d.ap())

                norm2_all = pb.tile([128, 6], f32, tag="norm2", name="norm2")
                scr_i = [0]

                def scr(tag, w=CH, dt=bf16, bufs=2):
                    scr_i[0] += 1
                    return pb.tile([128, w], dt, tag=tag, bufs=bufs,
                                   name=f"{tag}{scr_i[0]}")

                def conv_u(t9, u, u_chunks):
                    for (s0, s1) in u_chunks:
                        n = s1 - s0
                        psA = psB.tile([128, CH], f32, tag="conv", bufs=2,
                                       name=f"psA{t9}_{s0}")
                        for kk in range(3):
                            nc.tensor.matmul(
                                psA[:, :n],
                                lhsT=wT_sb[kk][:, 128 * t9:128 * t9 + 128],
                                rhs=x_sb[kk][:, s0:s1],
                                start=(kk == 0), stop=(kk == 2))
                        nc.scalar.copy(u[:, PAD + s0:PAD + s1], psA[:, :n])

                def dw_chunks(t9, u, dest, chunks, dst_off, pat, dwd):
                    """depthwise 3x3 of u into dest, per-chunk engine split."""
                    for ci, (s0, s1) in enumerate(chunks):
                        n = s1 - s0
                        cls = pat[ci]
                        do = dest[:, s0 + dst_off:s1 + dst_off]
                        if cls == 'pe':
                            psD = psB.tile([128, CH], f32, tag="dw", bufs=2,
                                           name=f"psD{t9}_{s0}")
                            for tap in range(9):
                                dy, dx = tap // 3 - 1, tap % 3 - 1
                                off = PAD + s0 + dy * COLS + dx
                                nc.tensor.matmul(
                                    psD[:, :n], lhsT=dwd[tap],
                                    rhs=u[:, off:off + n],
                                    start=(tap == 0), stop=(tap == 8))
                            nc.scalar.copy(do, psD[:, :n])
                        elif cls == 'dve':
                            for tap in range(9):
                                dy, dx = tap // 3 - 1, tap % 3 - 1
                                src = u[:, PAD + s0 + dy * COLS + dx:
                                        PAD + s0 + dy * COLS + dx + n]
                                if tap == 0:
                                    nc.vector.tensor_scalar_mul(
                                        do, src, dwcT[t9][:, tap:tap + 1])
                                else:
                                    t2 = scr("dvt")
                                    nc.vector.tensor_scalar_mul(
                                        t2[:, :n], src,
                                        dwcT[t9][:, tap:tap + 1])
                                    nc.vector.tensor_add(do, do, t2[:, :n])
                        else:  # mix: ACT muls + DVE adds
                            for tap in range(9):
                                dy, dx = tap // 3 - 1, tap % 3 - 1
                                src = u[:, PAD + s0 + dy * COLS + dx:
                                        PAD + s0 + dy * COLS + dx + n]
                                if tap == 0:
                                    nc.scalar.mul(do, src,
                                                  dwcT[t9][:, tap:tap + 1])
                                else:
                                    t2 = scr("mxt")
                                    nc.scalar.mul(t2[:, :n], src,
                                                  dwcT[t9][:, tap:tap + 1])
                                    nc.vector.tensor_add(do, do, t2[:, :n])

                # ---- qk tiles (dest holds only rows 3..66) ----
                u = pb.tile([128, FLAT], bf16, tag="u", bufs=1, name="u_t")
                nc.vector.memset(u[:, 0:PAD + 2 * COLS], 0.0)
                nc.vector.memset(u[:, PAD + 68 * COLS:FLAT], 0.0)
                for t9 in range(6):
                    if t9 == 0:
                        dwd = dwd_t0
                    else:
                        dwd = []
                        for tap in range(9):
                            dt_ = pb.tile([128, 128], bf16, tag="dwd", bufs=12,
                                          name=f"dwd{t9}_{tap}")
                            nc.sync.dma_start(dt_, dwd_d.ap()[t9, tap])
                            dwd.append(dt_)
                    conv_u(t9, u, qk_u_chunks)
                    dest = pb.tile([128, 64 * COLS], bf16, tag="qkst", bufs=1,
                                   name=f"qkst{t9}")
                    dvv = dest.rearrange("p (r c) -> p r c", c=COLS)
                    sqacc = pb.tile([128, 4], f32, tag="sqacc", bufs=2,
                                    name=f"sqa{t9}")
                    sqj = scr("sqj", w=2048, bufs=1)

                    def spill_half(hf):
                        # spill + l2norm squares per 32-row half so the
                        # next tile's WAR on dest clears early
                        nc.sync.dma_start(
                            qk_dram[t9, :, 4096 * hf:4096 * hf + 4096],
                            dvv[:, 32 * hf:32 * hf + 32, 1:129])
                        for cj in range(2):
                            ci = 2 * hf + cj
                            nc.scalar.activation(
                                sqj.rearrange("p (r c) -> p r c", r=16),
                                dvv[:, 16 * ci:16 * ci + 16, 1:129],
                                ACT.Square, accum_out=sqacc[:, ci:ci + 1])

                    dw_chunks(t9, u, dest, qk_dw_chunks[:9], -3 * COLS,
                              QK_PAT[:9], dwd)
                    spill_half(0)
                    dw_chunks(t9, u, dest, qk_dw_chunks[9:], -3 * COLS,
                              QK_PAT[9:], dwd)
                    spill_half(1)
                    nc.vector.tensor_reduce(
                        norm2_all[:, t9:t9 + 1], sqacc, axis=AX, op=ADD)
                    if t9 == 5:
                        nc.sync.dma_start(n2_dram.transpose([1, 0]), norm2_all)

                # preload the v-tile taps so B-v never waits behind the
                # gram transposes on the sync queue
                dwdv = [[None] * 9 for _ in range(3)]
                for vt in range(3):
                    for tap in range(9):
                        dt_ = pb.tile([128, 128], bf16, tag="dwdv", bufs=27,
                                      name=f"dwdv{vt}_{tap}")
                        nc.sync.dma_start(dt_, dwd_d.ap()[6 + vt, tap])
                        dwdv[vt][tap] = dt_

                # ---- gram: xbar-transposed reload + col-tiled matmuls ----
                G_ps = psB.tile([128, 256], f32, tag="gram", bufs=1,
                                name="G_ps")
                tr_n = [0]

                def gram_groups(g_lo, g_hi):
                    for g in range(g_lo, g_hi):
                        stg = pb.tile([128, 4, 6, 128], bf16, tag="stage",
                                      bufs=2, name=f"stg{g}")
                        for t in range(6):
                            eng = (nc.scalar
                                   if tr_n[0] % TR_SCALAR_MOD == TR_SCALAR_MOD - 1
                                   else nc.sync)
                            tr_n[0] += 1
                            eng.dma_start_transpose(
                                stg[:, :, t, :],
                                qk_dram[t, :, CH * g:CH * g + CH])
                        stv = stg.rearrange("p a t c -> p a (t c)")
                        for m in range(4):
                            for h in range(HEADS):
                                base = (h % 2) * 64
                                qc = stv[:, m, 48 * h:48 * h + 48]
                                kc = stv[:, m, 384 + 48 * h:384 + 48 * h + 48]
                                nc.tensor.matmul(
                                    G_ps[base:base + 48,
                                         64 * (h // 2):64 * (h // 2) + 48],
                                    lhsT=kc, rhs=qc,
                                    start=(g == 0 and m == 0 and h < 2),
                                    stop=(g == 15 and m == 3 and h >= 6),
                                    tile_position=(0, base),
                                    skip_group_check=True)

                # ---- v tiles (overlap the gram drain) ----
                gram_slices = [(0, 6), (6, 12), (12, 16)]
                for t9 in range(6, 9):
                    vt = t9 - 6
                    dest = pp.tile([128, FLAT], bf16, tag=f"v{vt}",
                                   name=f"v{vt}")
                    v_sb[vt] = dest
                    conv_u(t9, u, full_chunks)
                    dw_chunks(t9, u, dest, v_dw_chunks, PAD, V_PAT, dwdv[vt])
                    vv = dest[:, PAD:PAD + NPOS].rearrange(
                        "p (r c) -> p r c", c=COLS)
                    nc.vector.memset(vv[:, 1:69, 0:1], 0.0)
                    nc.vector.memset(vv[:, 1:69, COLS - 1:COLS], 0.0)
                    nc.vector.tensor_scalar_mul(
                        vv[:, 2], vv[:, 2], ez_sb[:, 0:1])
                    nc.vector.tensor_scalar_mul(
                        vv[:, 67], vv[:, 67], ez_sb[:, 1:2])
                    gram_groups(*gram_slices[vt])

                # ---- payload: gram blocks + l2norm sums ----
                Gsbv = G_sb.rearrange("p (b c) -> p b c", b=HEADS)
                nc.vector.tensor_copy(
                    Gsbv[0:48, 0:4, 0:48],
                    G_ps[0:48].rearrange("p (j c) -> p j c", j=4)[:, :, 0:48])
                nc.vector.tensor_copy(
                    Gsbv[0:48, 4:8, 0:48],
                    G_ps[64:112].rearrange("p (j c) -> p j c", j=4)[:, :, 0:48])
                n2flat = n2_dram.rearrange("t p -> (t p)").rearrange(
                    "(u h c) -> u h c", u=2, h=HEADS)
                # payload col 48 = qnorm2, col 49 = knorm2; head h -> block
                # (h%2)*4 + h//2, i.e. src head rows reordered by parity
                for u2 in range(2):
                    for par in range(2):
                        src = n2flat[u2].rearrange(
                            "(j p2) c -> j p2 c", p2=2)[:, par]
                        nc.sync.dma_start(
                            Gsbv[:, 4 * par:4 * par + 4, 48 + u2:49 + u2].opt(),
                            src.transpose([1, 0]).opt())

                # ================= AllReduce =================
                nc.sync.dma_start(cc_in, G_sb)
                nc.gpsimd.collective_compute(
                    "AllReduce", ADD,
                    replica_groups=[[0, 1], [2, 3], [4, 5], [6, 7]],
                    ins=[cc_in.opt()], outs=[cc_out.opt()])
                nc.sync.dma_start(G_sb, cc_out)

                # ============ softmax / attn / M build (early) ============
                Gv = G_sb.rearrange("p (b c) -> p b c", b=HEADS)
                nrm = pb.tile([48, 16], f32, tag="nrm", name="nrm")
                inv = pb.tile([48, 16], f32, tag="inv", name="inv")
                # cols 2b = qnorm, 2b+1 = knorm (payload-block order)
                nc.scalar.sqrt(
                    nrm.rearrange("p (b u) -> p b u", b=HEADS),
                    Gv[:, :, 48:50])
                nc.vector.tensor_scalar_max(nrm, nrm, 1e-12)
                nc.vector.reciprocal(inv, nrm)

                for h in range(HEADS):
                    b = blk(h)
                    B = pb.tile([48, 48], f32, tag="B", bufs=2, name=f"B{h}")
                    nc.vector.tensor_scalar(
                        out=B, in0=Gv[:, b, 0:48],
                        scalar1=inv[:, 2 * b + 1:2 * b + 2],
                        scalar2=float(temp[h]),
                        op0=MM, op1=MM)
                    psb = psB.tile([48, 48], f32, tag="ptrE", bufs=2,
                                   name=f"psb{h}")
                    nc.tensor.transpose(psb, B, id48_sb)
                    A0 = pb.tile([48, 48], f32, tag="A0", bufs=2,
                                 name=f"A0{h}")
                    nc.vector.tensor_scalar_mul(
                        A0, psb, inv[:, 2 * b:2 * b + 1])
                    e0 = pb.tile([48, 48], f32, tag="e0", bufs=2,
                                 name=f"e0{h}")
                    s_ = pb.tile([48, 4], f32, tag="s", bufs=2, name=f"s{h}")
                    nc.scalar.activation(e0, A0, ACT.Exp,
                                         accum_out=s_[:, 0:1])
                    e1 = pb.tile([48, 48], f32, tag="e1", bufs=2,
                                 name=f"e1{h}")
                    nc.vector.tensor_mul(e1, e0,
                                         mm_sb[:, 48 * h:48 * h + 48])
                    nc.vector.tensor_reduce(
                        s_[:, 1:2], e1, axis=AX, op=ADD)
                    r_ = pb.tile([48, 4], f32, tag="r", bufs=2, name=f"r{h}")
                    nc.vector.reciprocal(r_[:, 0:2], s_[:, 0:2])
                    nc.vector.tensor_scalar_mul(r_[:, 0:1], r_[:, 0:1],
                                                float(wsm[0]))
                    nc.vector.tensor_scalar_mul(r_[:, 1:2], r_[:, 1:2],
                                                float(wsm[1]))
                    t0 = pb.tile([48, 48], f32, tag="t0", bufs=2,
                                 name=f"t0{h}")
                    nc.vector.tensor_scalar_mul(t0, e0, r_[:, 0:1])
                    af = pb.tile([48, 48], f32, tag="af", bufs=2,
                                 name=f"af{h}")
                    nc.vector.tensor_scalar_mul(af, e1, r_[:, 1:2])
                    nc.vector.tensor_add(af, af, t0)
                    nc.vector.tensor_copy(attn_bf[h], af)

                # M = proj @ blockdiag(attn), transposed into MT_sb
                M_sb = []
                for ot in range(3):
                    M_ps = psB.tile([128, DIM], f32, tag="mps", bufs=1,
                                    name=f"mps{ot}")
                    for h in range(HEADS):
                        nc.tensor.matmul(
                            M_ps[:, 48 * h:48 * h + 48],
                            lhsT=projT_sb[h][:, 128 * ot:128 * ot + 128],
                            rhs=attn_bf[h], start=True, stop=True)
                    t = pb.tile([128, DIM], bf16, tag=f"msb{ot}",
                                name=f"msb{ot}")
                    nc.vector.tensor_copy(t, M_ps)
                    M_sb.append(t)
                for ot in range(3):
                    for dtt in range(3):
                        psM = psB.tile([128, 128], bf16, tag="ptrE",
                                       bufs=2, name=f"psM{ot}_{dtt}")
                        nc.tensor.transpose(
                            psM, M_sb[ot][:, 128 * dtt:128 * dtt + 128],
                            ident_sb)
                        nc.vector.tensor_copy(
                            MT_sb[dtt][:, 128 * ot:128 * ot + 128], psM)

            # ============ late phase: pos path + fused out ============
            with tc.tile_pool(name="late", bufs=1) as lp, \
                 tc.tile_pool(name="psL", bufs=1, space="PSUM") as psL:
                g1_sb = [lp.tile([128, FLAT], bf16, tag=f"g1_{i}",
                                 name=f"g1_{i}") for i in range(3)]
                # pos-path taps; issue on scalar to stay clear of any
                # remaining sync-queue work
                dwd1_sb = [[None] * 9 for _ in range(3)]
                dwd2_sb = [[None] * 9 for _ in range(3)]
                for vt in range(3):
                    for tap in range(9):
                        t1 = lp.tile([128, 128], bf16, tag="dwp1", bufs=27,
                                     name=f"dwp1_{vt}_{tap}")
                        nc.sync.dma_start(t1, dwd_d.ap()[9 + vt, tap])
                        dwd1_sb[vt][tap] = t1
                        t2 = lp.tile([128, 128], bf16, tag="dwp2", bufs=27,
                                     name=f"dwp2_{vt}_{tap}")
                        nc.sync.dma_start(t2, dwd_d.ap()[12 + vt, tap])
                        dwd2_sb[vt][tap] = t2
                lscr_i = [0]

                def lscr(tag):
                    lscr_i[0] += 1
                    return lp.tile([128, CH], bf16, tag=tag, bufs=3,
                                   name=f"{tag}{lscr_i[0]}")

                def dw1_tile(vt):
                    g1 = g1_sb[vt]
                    nc.vector.memset(g1[:, 0:PAD + 2 * COLS], 0.0)
                    nc.vector.memset(g1[:, PAD + 68 * COLS:FLAT], 0.0)
                    for ci, (s0, s1) in enumerate(dw1_chunks):
                        n = s1 - s0
                        cls = DW1_PAT[ci]
                        if cls == 'pe':
                            psP = psL.tile([128, CH], f32, tag="dwg", bufs=2,
                                           name=f"psP{vt}_{s0}")
                            for tap in range(9):
                                dy, dx = tap // 3 - 1, tap % 3 - 1
                                off = PAD + s0 + dy * COLS + dx
                                nc.tensor.matmul(
                                    psP[:, :n], lhsT=dwd1_sb[vt][tap],
                                    rhs=v_sb[vt][:, off:off + n],
                                    start=(tap == 0), stop=(tap == 8))
                            nc.scalar.activation(g1[:, PAD + s0:PAD + s1],
                                                 psP[:, :n], ACT.Gelu)
                            continue
                        acc = lscr("dacc")
                        for tap in range(9):
                            dy, dx = tap // 3 - 1, tap % 3 - 1
                            src = v_sb[vt][:, PAD + s0 + dy * COLS + dx:
                                           PAD + s0 + dy * COLS + dx + n]
                            wap = dwcT[9 + vt][:, tap:tap + 1]
                            if tap == 0:
                                if cls == 'dve':
                                    nc.vector.tensor_scalar_mul(
                                        acc[:, :n], src, wap)
                                else:
                                    nc.scalar.mul(acc[:, :n], src, wap)
                            else:
                                t2 = lscr("lmx")
                                if cls == 'dve':
                                    nc.vector.tensor_scalar_mul(
                                        t2[:, :n], src, wap)
                                else:
                                    nc.scalar.mul(t2[:, :n], src, wap)
                                nc.vector.tensor_add(acc[:, :n], acc[:, :n],
                                                     t2[:, :n])
                        nc.scalar.activation(g1[:, PAD + s0:PAD + s1],
                                             acc[:, :n], ACT.Gelu)
                    g1v = g1[:, PAD:PAD + NPOS].rearrange(
                        "p (r c) -> p r c", c=COLS)
                    nc.vector.memset(g1v[:, 2:68, 0:1], 0.0)
                    nc.vector.memset(g1v[:, 2:68, COLS - 1:COLS], 0.0)
                    nc.vector.tensor_scalar_mul(
                        g1v[:, 2], g1v[:, 2], ez_sb[:, 0:1])
                    nc.vector.tensor_scalar_mul(
                        g1v[:, 67], g1v[:, 67], ez_sb[:, 1:2])

                vviews = [v_sb[i][:, PAD:PAD + NPOS].rearrange(
                    "p (r c) -> p r c", c=COLS) for i in range(3)]

                def out_tile(ot):
                    g1view = g1_sb[ot][:, PAD:PAD + NPOS].rearrange(
                        "p (r c) -> p r c", c=COLS)
                    for i in range(16):
                        psO = psL.tile([128, CH], f32, tag="out", bufs=2,
                                       name=f"psO{ot}_{i}")
                        for tap in range(9):
                            dy, dx = tap // 3 - 1, tap % 3 - 1
                            rhs = g1view[:, 3 + 4 * i + dy:7 + 4 * i + dy,
                                         1 + dx:129 + dx]
                            nc.tensor.matmul(
                                psO, lhsT=dwd2_sb[ot][tap], rhs=rhs,
                                start=(tap == 0), stop=False)
                        for dtt in range(3):
                            rhs = vviews[dtt][:, 3 + 4 * i:7 + 4 * i, 1:129]
                            nc.tensor.matmul(
                                psO,
                                lhsT=MT_sb[dtt][:, 128 * ot:128 * ot + 128],
                                rhs=rhs, start=False, stop=(dtt == 2))
                        outf = lp.tile([128, CH], f32, tag="outf", bufs=3,
                                       name=f"outf{ot}_{i}")
                        nc.scalar.copy(outf, psO)
                        nc.sync.dma_start(
                            out_d.ap()[128 * ot:128 * ot + 128,
                                       CH * i:CH * i + CH], outf)

                for i in range(3):
                    dw1_tile(i)
                    out_tile(i)

    nc.compile()
    return nc


def _host_prep(x, mask, qkv_w, dw_w, proj_w, temperature, w_blend,
               pos_w1, pos_w2):
    x = np.asarray(x, np.float32)
    b = x.shape[0]
    xp = np.zeros((b, DIM, 134, COLS), np.float32)
    xp[:, :, 3:131, 1:129] = x
    shards = []
    for core in range(NCORES):
        bi, s = core // 2, core % 2
        shards.append(np.ascontiguousarray(
            xp[bi, :, 64 * s:64 * s + ROWS, :]).reshape(DIM, NPOS).astype(BF))
    wT = np.ascontiguousarray(
        np.asarray(qkv_w, np.float32)[:, :, 0, 0].T).astype(BF)
    dwd = np.zeros((15, 9, 128, 128), np.float32)
    dwk = np.asarray(dw_w, np.float32)[:, 0]       # (1152, 3, 3)
    pk1 = np.asarray(pos_w1, np.float32)[:, 0]     # (384, 3, 3)
    pk2 = np.asarray(pos_w2, np.float32)[:, 0]
    idx = np.arange(128)
    for t9 in range(9):
        for tap in range(9):
            dwd[t9, tap, idx, idx] = dwk[128 * t9:128 * t9 + 128,
                                         tap // 3, tap % 3]
    for vt in range(3):
        for tap in range(9):
            dwd[9 + vt, tap, idx, idx] = pk1[128 * vt:128 * vt + 128,
                                             tap // 3, tap % 3]
            dwd[12 + vt, tap, idx, idx] = pk2[128 * vt:128 * vt + 128,
                                              tap // 3, tap % 3]
    dwcol = np.zeros((15, 9, 128), np.float32)
    for t in range(15):
        for tap in range(9):
            dwcol[t, tap] = dwd[t, tap, idx, idx]
    dwd = dwd.astype(BF)
    pw = np.asarray(proj_w, np.float32)[:, :, 0, 0]
    projT = np.stack([np.ascontiguousarray(pw[:, 48 * h:48 * h + 48].T)
                      for h in range(HEADS)]).astype(BF)     # (8, 48, 384)
    mmul = (np.asarray(mask)[0] != 0).astype(np.float32)     # (8, 48, 48)
    maskmul = np.ascontiguousarray(
        mmul.transpose(1, 0, 2).reshape(48, HEADS * 48))
    ident = np.eye(128, dtype=np.float32).astype(BF)
    ident48 = np.eye(48, dtype=np.float32)
    wb = np.asarray(w_blend, np.float32)
    e = np.exp(wb - wb.max())
    wsm = e / e.sum()
    temp = np.asarray(temperature, np.float32).reshape(HEADS)
    return (shards, wT, dwd, dwcol, projT, maskmul, ident, ident48, wsm, temp)


def kernel(**inputs):
    from concourse import bass_utils
    (shards, wT, dwd, dwcol, projT, maskmul, ident, ident48, wsm,
     temp) = _host_prep(**inputs)
    key = (tuple(np.round(wsm, 8)), tuple(np.round(temp, 8)))
    if key not in _CACHE:
        _CACHE[key] = _build(wsm, temp)
    nc = _CACHE[key]
    in_maps = []
    for core in range(NCORES):
        s = core % 2
        ez = np.ones((128, 2), np.float32)
        ez[:, 0] = 0.0 if s == 0 else 1.0
        ez[:, 1] = 0.0 if s == 1 else 1.0
        in_maps.append({
            "x": shards[core], "wT": wT, "dwd": dwd, "dwcol": dwcol,
            "projT": projT, "maskmul": maskmul, "ident": ident,
            "ident48": ident48, "ez": ez,
        })
    res = bass_utils.run_bass_kernel_spmd(
        nc, in_maps, core_ids=list(range(NCORES)),
        trace=bool(int(__import__("os").environ.get("KBENCH_TRACE", "0"))))
    kernel._last_result = res
    x = np.asarray(inputs["x"])
    out = np.zeros((x.shape[0], DIM, 128, 128), np.float32)
    for core in range(NCORES):
        bi, s = core // 2, core % 2
        out[bi, :, 64 * s:64 * s + 64, :] = \
            np.asarray(res.results[core]["out"], np.float32).reshape(
                DIM, 64, 128)
    return out


# revision 21
# speedup vs baseline: 1.0317x; 1.0317x over previous
"""Trainium2 Bass kernel for nn_Attention_12695923327433 (8-core SPMD).

Sharding: batch(4) x H-strips(2) -> 8 cores. Each core computes a
(384, 64, 128) slice of the output. Cross-core comm: one tiny AllReduce of
per-head gram matrices (for the l2norm + q@k attention logits) between the
two strip-cores of each batch sample.

v2 schedule. The depthwise 3x3 convs dominate; each 512-col chunk is
assigned to one of three engine classes to balance occupancy:
  'pe'  9 PSUM-accumulated diagonal matmuls + ACT cast,
  'dve' 9 tensor_scalar muls + 8 tensor_tensor adds in bf16,
  'mix' 9 ACT per-partition muls + 8 DVE adds.
Other deltas vs v1: x streams in kk-interleaved column blocks so the
first matmul starts early; gram transposes are split across the sync
and scalar HWDGE queues; softmax + M-build run right after the
AllReduce inside phase B so the out stage never waits on attention
weights; pos-dw1 tiles are interleaved with the fused out chunks.
"""
import sys
sys.path.insert(0, "/opt/trn_rl_repo")
import numpy as np
import ml_dtypes

BF = ml_dtypes.bfloat16
DIM, HEADS, NCORES = 384, 8, 8
ROWS, COLS = 70, 130          # 3+64+3 rows, 1+128+1 cols (zero-padded halo)
NPOS = ROWS * COLS            # 9100
PAD = 132                     # flat guard so shifted APs stay in-bounds
FLAT = PAD + NPOS + PAD
CH = 512

_CACHE = {}

# engine class per dw chunk index (tuned for per-phase engine balance)
QK_PAT = ['pe', 'dve', 'pe', 'dve', 'pe', 'dve', 'pe', 'mix', 'pe',
          'dve', 'pe', 'dve', 'pe', 'dve', 'pe', 'pe', 'pe']       # 17
V_PAT = ['pe', 'dve', 'pe', 'dve', 'pe', 'dve', 'pe', 'dve', 'pe',
         'dve', 'pe', 'dve', 'pe', 'dve', 'pe', 'pe', 'pe', 'pe']  # 18
DW1_PAT = ['dve', 'mix', 'dve', 'pe', 'dve', 'mix', 'dve', 'pe', 'dve',
           'mix', 'dve', 'pe', 'dve', 'dve', 'pe', 'pe', 'dve']    # 17
TR_SCALAR_MOD = 6             # every 6th gram transpose issues on nc.scalar


def _build(wsm, temp):
    from concourse import bass, bacc, tile, mybir

    f32 = mybir.dt.float32
    bf16 = mybir.dt.bfloat16
    MM = mybir.AluOpType.mult
    ADD = mybir.AluOpType.add
    AX = mybir.AxisListType.X
    ACT = mybir.ActivationFunctionType

    nc = bacc.Bacc("TRN2", target_bir_lowering=False, debug=False,
                   num_devices=NCORES)

    x_d = nc.dram_tensor("x", [DIM, NPOS], bf16, kind="ExternalInput")
    wT_d = nc.dram_tensor("wT", [DIM, 3 * DIM], bf16, kind="ExternalInput")
    dwd_d = nc.dram_tensor("dwd", [15, 9, 128, 128], bf16, kind="ExternalInput")
    projT_d = nc.dram_tensor("projT", [HEADS, 48, DIM], bf16, kind="ExternalInput")
    mm_d = nc.dram_tensor("maskmul", [48, HEADS * 48], f32, kind="ExternalInput")
    id_d = nc.dram_tensor("ident", [128, 128], bf16, kind="ExternalInput")
    id48_d = nc.dram_tensor("ident48", [48, 48], f32, kind="ExternalInput")
    ez_d = nc.dram_tensor("ez", [128, 2], f32, kind="ExternalInput")
    dwc_d = nc.dram_tensor("dwcol", [15, 9, 128], f32, kind="ExternalInput")
    out_d = nc.dram_tensor("out", [DIM, 64 * 128], f32, kind="ExternalOutput")

    # flat-region chunking (rows of the 70x130 halo grid, 512-wide chunks)
    full_chunks = [(s, min(NPOS, s + CH)) for s in range(0, NPOS, CH)]
    qk_u_chunks = [(s, min(68 * COLS, s + CH))
                   for s in range(2 * COLS, 68 * COLS, CH)]
    qk_dw_chunks = [(s, min(67 * COLS, s + CH))
                    for s in range(3 * COLS, 67 * COLS, CH)]
    v_dw_chunks = [(s, min(69 * COLS, s + CH))
                   for s in range(1 * COLS, 69 * COLS, CH)]
    dw1_chunks = [(s, min(68 * COLS, s + CH))
                  for s in range(2 * COLS, 68 * COLS, CH)]

    # payload block for head h (even heads on psum partitions 0-47,
    # odd heads on 64-111; payload blocks even-first)
    def blk(h):
        return (h % 2) * 4 + h // 2

    with tile.TileContext(nc) as tc:
        with tc.tile_pool(name="const", bufs=1) as cp, \
             tc.tile_pool(name="persist", bufs=1) as pp, \
             tc.tile_pool(name="dramp", bufs=1, space="DRAM") as dp:

            # ---- constants (before x so compute starts early) ----
            wT_sb = []
            for kk in range(3):
                t = cp.tile([128, 3 * DIM], bf16, tag=f"wT{kk}", name=f"wT{kk}")
                nc.sync.dma_start(t, wT_d.ap()[128 * kk:128 * kk + 128, :])
                wT_sb.append(t)
            dwcT = [cp.tile([128, 9], f32, tag=f"dwc{t9}", name=f"dwc{t9}")
                    for t9 in range(15)]
            ez_sb = cp.tile([128, 2], f32, tag="ez", name="ez")

            G_sb = pp.tile([48, 2048], f32, tag="gsb", name="G_sb")
            attn_bf = [pp.tile([48, 48], bf16, tag=f"at{h}", name=f"atl{h}")
                       for h in range(HEADS)]
            MT_sb = [pp.tile([128, DIM], bf16, tag=f"mt{i}", name=f"mtl{i}")
                     for i in range(3)]
            v_sb = [None] * 3

            qk_dram = dp.tile([6, 128, 64 * 128], bf16, tag="qkspill",
                              name="qk_dram")
            n2_dram = dp.tile([6, 128], f32, tag="n2", name="n2")
            cc_in = dp.tile([48, 2048], f32, tag="ccin", name="ccin")
            cc_out = dp.tile([48, 2048], f32, tag="ccout", name="ccout")

            # ================= Phase B + gram =================
            with tc.tile_pool(name="pb", bufs=1) as pb, \
                 tc.tile_pool(name="psB", bufs=1, space="PSUM") as psB:
                x_sb = [pb.tile([128, NPOS], bf16, tag=f"x{kk}", name=f"x{kk}")
                        for kk in range(3)]
                # first x blocks right after wT so the first conv starts
                # early; tile-0 taps and the rest of the constants follow
                dwd_t0 = []
                NB = 8
                for b6 in range(NB):
                    s = (NPOS // NB) * b6
                    e = NPOS if b6 == NB - 1 else (NPOS // NB) * (b6 + 1)
                    for kk in range(3):
                        nc.sync.dma_start(x_sb[kk][:, s:e],
                                          x_d.ap()[128 * kk:128 * kk + 128, s:e])
                    if b6 == 0:
                        for tap in range(9):
                            dt_ = pb.tile([128, 128], bf16, tag="dwd",
                                          bufs=12, name=f"dwd0_{tap}")
                            nc.sync.dma_start(dt_, dwd_d.ap()[0, tap])
                            dwd_t0.append(dt_)
                    if b6 == 1:
                        for t15 in range(15):
                            nc.sync.dma_start(
                                dwcT[t15], dwc_d.ap()[t15].transpose([1, 0]))
                        nc.sync.dma_start(ez_sb, ez_d.ap())
                    if b6 == 3:
                        # late-consumed constants ride mid-stream
                        projT_sb = []
                        for h in range(HEADS):
                            t = cp.tile([48, DIM], bf16, tag=f"pjT{h}",
                                        name=f"pjT{h}")
                            nc.sync.dma_start(t, projT_d.ap()[h])
                            projT_sb.append(t)
                        ident_sb = cp.tile([128, 128], bf16, tag="ident",
                                           name="ident")
                        nc.sync.dma_start(ident_sb, id_d.ap())
                        id48_sb = cp.tile([48, 48], f32, tag="id48",
                                          name="id48")
                        nc.sync.dma_start(id48_sb, id48_d.ap())
                        mm_sb = cp.tile([48, HEADS * 48], f32, tag="mm",
                                        name="mm")
                        nc.sync.dma_start(mm_sb, mm_d.ap())

                norm2_all = pb.tile([128, 6], f32, tag="norm2", name="norm2")
                scr_i = [0]

                def scr():
                    scr_i[0] += 1
                    return pb.tile([128, CH], bf16, tag="dvt", bufs=3,
                                   name=f"dvt{scr_i[0]}")

                def conv_u(t9, u, u_chunks):
                    for (s0, s1) in u_chunks:
                        n = s1 - s0
                        psA = psB.tile([128, CH], f32, tag="conv", bufs=2,
                                       name=f"psA{t9}_{s0}")
                        for kk in range(3):
                            nc.tensor.matmul(
                                psA[:, :n],
                                lhsT=wT_sb[kk][:, 128 * t9:128 * t9 + 128],
                                rhs=x_sb[kk][:, s0:s1],
                                start=(kk == 0), stop=(kk == 2))
                        nc.scalar.copy(u[:, PAD + s0:PAD + s1], psA[:, :n])

                def dw_chunks(t9, u, dest, chunks, dst_off, pat, dwd):
                    """depthwise 3x3 of u into dest, per-chunk engine split."""
                    for ci, (s0, s1) in enumerate(chunks):
                        n = s1 - s0
                        cls = pat[ci]
                        do = dest[:, s0 + dst_off:s1 + dst_off]
                        if cls == 'pe':
                            psD = psB.tile([128, CH], f32, tag="dw", bufs=2,
                                           name=f"psD{t9}_{s0}")
                            for tap in range(9):
                                dy, dx = tap // 3 - 1, tap % 3 - 1
                                off = PAD + s0 + dy * COLS + dx
                                nc.tensor.matmul(
                                    psD[:, :n], lhsT=dwd[tap],
                                    rhs=u[:, off:off + n],
                                    start=(tap == 0), stop=(tap == 8))
                            nc.scalar.copy(do, psD[:, :n])
                        elif cls == 'dve':
                            for tap in range(9):
                                dy, dx = tap // 3 - 1, tap % 3 - 1
                                src = u[:, PAD + s0 + dy * COLS + dx:
                                        PAD + s0 + dy * COLS + dx + n]
                                if tap == 0:
                                    nc.vector.tensor_scalar_mul(
                                        do, src, dwcT[t9][:, tap:tap + 1])
                                else:
                                    t2 = scr()
                                    nc.vector.tensor_scalar_mul(
                                        t2[:, :n], src,
                                        dwcT[t9][:, tap:tap + 1])
                                    nc.vector.tensor_add(do, do, t2[:, :n])
                        else:  # mix: ACT muls + DVE adds
                            for tap in range(9):
                                dy, dx = tap // 3 - 1, tap % 3 - 1
                                src = u[:, PAD + s0 + dy * COLS + dx:
                                        PAD + s0 + dy * COLS + dx + n]
                                if tap == 0:
                                    nc.scalar.mul(do, src,
                                                  dwcT[t9][:, tap:tap + 1])
                                else:
                                    t2 = scr()
                                    nc.scalar.mul(t2[:, :n], src,
                                                  dwcT[t9][:, tap:tap + 1])
                                    nc.vector.tensor_add(do, do, t2[:, :n])

                # ---- qk tiles (dest holds only rows 3..66) ----
                u = pb.tile([128, FLAT], bf16, tag="u", bufs=1, name="u_t")
                nc.vector.memset(u[:, 0:PAD + 2 * COLS], 0.0)
                nc.vector.memset(u[:, PAD + 68 * COLS:FLAT], 0.0)
                for t9 in range(6):
                    if t9 == 0:
                        dwd = dwd_t0
                    else:
                        dwd = []
                        for tap in range(9):
                            dt_ = pb.tile([128, 128], bf16, tag="dwd", bufs=12,
                                          name=f"dwd{t9}_{tap}")
                            nc.sync.dma_start(dt_, dwd_d.ap()[t9, tap])
                            dwd.append(dt_)
                    conv_u(t9, u, qk_u_chunks)
                    # qkst sized 68*COLS so the tag can host g1_0 later
                    dest = pb.tile([128, 68 * COLS], bf16, tag="qkst", bufs=1,
                                   name=f"qkst{t9}")
                    dvv = dest[:, :64 * COLS].rearrange(
                        "p (r c) -> p r c", c=COLS)
                    sqacc = pb.tile([128, 4], f32, tag="sqacc", bufs=2,
                                    name=f"sqa{t9}")
                    sqj = pb.tile([128, 2048], bf16, tag="stage", bufs=2,
                                  name=f"sqj{t9}")

                    def spill_half(hf):
                        # spill + l2norm squares per 32-row half so the
                        # next tile's WAR on dest clears early
                        nc.sync.dma_start(
                            qk_dram[t9, :, 4096 * hf:4096 * hf + 4096],
                            dvv[:, 32 * hf:32 * hf + 32, 1:129])
                        for cj in range(2):
                            ci = 2 * hf + cj
                            nc.scalar.activation(
                                sqj.rearrange("p (r c) -> p r c", r=16),
                                dvv[:, 16 * ci:16 * ci + 16, 1:129],
                                ACT.Square, accum_out=sqacc[:, ci:ci + 1])

                    dw_chunks(t9, u, dest, qk_dw_chunks[:9], -3 * COLS,
                              QK_PAT[:9], dwd)
                    spill_half(0)
                    dw_chunks(t9, u, dest, qk_dw_chunks[9:], -3 * COLS,
                              QK_PAT[9:], dwd)
                    spill_half(1)
                    nc.vector.tensor_reduce(
                        norm2_all[:, t9:t9 + 1], sqacc, axis=AX, op=ADD)
                    if t9 == 5:
                        nc.sync.dma_start(n2_dram.transpose([1, 0]), norm2_all)

                # preload the v-tile taps so B-v never waits behind the
                # gram transposes on the sync queue
                dwdv = [[None] * 9 for _ in range(3)]
                for vt in range(3):
                    for tap in range(9):
                        dt_ = pb.tile([128, 128], bf16, tag="dwdv", bufs=27,
                                      name=f"dwdv{vt}_{tap}")
                        nc.sync.dma_start(dt_, dwd_d.ap()[6 + vt, tap])
                        dwdv[vt][tap] = dt_

                # ---- gram: xbar-transposed reload + col-tiled matmuls ----
                G_ps = psB.tile([128, 256], f32, tag="gram", bufs=1,
                                name="G_ps")
                tr_n = [0]

                def gram_groups(g_lo, g_hi):
                    for g in range(g_lo, g_hi):
                        stg = pb.tile([128, 4, 6, 128], bf16, tag="stage",
                                      bufs=2, name=f"stg{g}")
                        for t in range(6):
                            eng = (nc.scalar
                                   if tr_n[0] % TR_SCALAR_MOD == TR_SCALAR_MOD - 1
                                   else nc.sync)
                            tr_n[0] += 1
                            eng.dma_start_transpose(
                                stg[:, :, t, :],
                                qk_dram[t, :, CH * g:CH * g + CH])
                        stv = stg.rearrange("p a t c -> p a (t c)")
                        for m in range(4):
                            for h in range(HEADS):
                                base = (h % 2) * 64
                                qc = stv[:, m, 48 * h:48 * h + 48]
                                kc = stv[:, m, 384 + 48 * h:384 + 48 * h + 48]
                                nc.tensor.matmul(
                                    G_ps[base:base + 48,
                                         64 * (h // 2):64 * (h // 2) + 48],
                                    lhsT=kc, rhs=qc,
                                    start=(g == 0 and m == 0 and h < 2),
                                    stop=(g == 15 and m == 3 and h >= 6),
                                    tile_position=(0, base),
                                    skip_group_check=True)

                # ---- v tiles (gram groups interleaved per 6-chunk segment
                # so the stg ring never head-of-line blocks the transposes)
                gram_sched = [[(0, 2), (2, 4), (4, 6)],
                              [(6, 8), (8, 10), (10, 12)],
                              [(12, 14), (14, 15), (15, 16)]]
                for t9 in range(6, 9):
                    vt = t9 - 6
                    dest = pp.tile([128, FLAT], bf16, tag=f"v{vt}",
                                   name=f"v{vt}")
                    v_sb[vt] = dest
                    conv_u(t9, u, full_chunks)
                    for seg in range(3):
                        lo, hi = 6 * seg, min(18, 6 * seg + 6)
                        dw_chunks(t9, u, dest, v_dw_chunks[lo:hi], PAD,
                                  V_PAT[lo:hi], dwdv[vt])
                        gram_groups(*gram_sched[vt][seg])
                    vv = dest[:, PAD:PAD + NPOS].rearrange(
                        "p (r c) -> p r c", c=COLS)
                    nc.vector.memset(vv[:, 1:69, 0:1], 0.0)
                    nc.vector.memset(vv[:, 1:69, COLS - 1:COLS], 0.0)
                    nc.vector.tensor_scalar_mul(
                        vv[:, 2], vv[:, 2], ez_sb[:, 0:1])
                    nc.vector.tensor_scalar_mul(
                        vv[:, 67], vv[:, 67], ez_sb[:, 1:2])

                # pos-path taps reuse the dwdv/dwd rings (v taps are dead);
                # issued before the collective so the sync queue never
                # holds them behind the AllReduce-gated reload
                dwd1_sb = [[None] * 9 for _ in range(3)]
                dwd2_sb = [[None] * 9 for _ in range(3)]
                for vt in range(3):
                    for tap in range(9):
                        t1 = pb.tile([128, 128], bf16, tag="dwdv", bufs=27,
                                     name=f"dwp1_{vt}_{tap}")
                        nc.sync.dma_start(t1, dwd_d.ap()[9 + vt, tap])
                        dwd1_sb[vt][tap] = t1
                for vt in range(3):
                    for tap in range(9):
                        t2 = pb.tile([128, 128], bf16, tag="dwd", bufs=12,
                                     name=f"dwp2_{vt}_{tap}")
                        nc.sync.dma_start(t2, dwd_d.ap()[12 + vt, tap])
                        dwd2_sb[vt][tap] = t2

                # ---- payload: gram blocks + l2norm sums ----
                Gsbv = G_sb.rearrange("p (b c) -> p b c", b=HEADS)
                nc.vector.tensor_copy(
                    Gsbv[0:48, 0:4, 0:48],
                    G_ps[0:48].rearrange("p (j c) -> p j c", j=4)[:, :, 0:48])
                nc.vector.tensor_copy(
                    Gsbv[0:48, 4:8, 0:48],
                    G_ps[64:112].rearrange("p (j c) -> p j c", j=4)[:, :, 0:48])
                n2flat = n2_dram.rearrange("t p -> (t p)").rearrange(
                    "(u h c) -> u h c", u=2, h=HEADS)
                # payload col 48 = qnorm2, col 49 = knorm2; head h -> block
                # (h%2)*4 + h//2, i.e. src head rows reordered by parity
                for u2 in range(2):
                    for par in range(2):
                        src = n2flat[u2].rearrange(
                            "(j p2) c -> j p2 c", p2=2)[:, par]
                        nc.sync.dma_start(
                            Gsbv[:, 4 * par:4 * par + 4, 48 + u2:49 + u2].opt(),
                            src.transpose([1, 0]).opt())

                # ================= AllReduce =================
                nc.sync.dma_start(cc_in, G_sb)
                nc.gpsimd.collective_compute(
                    "AllReduce", ADD,
                    replica_groups=[[0, 1], [2, 3], [4, 5], [6, 7]],
                    ins=[cc_in.opt()], outs=[cc_out.opt()])
                nc.sync.dma_start(G_sb, cc_out)

                # g1 tiles reuse the qkst / x tag space (dead by now);
                # g1 needs no halo guards: all reads lie in [260, 8840)
                g1_sb = [pb.tile([128, 68 * COLS], bf16, tag=t_, name=f"g1{t_}")
                         for t_ in ("qkst", "x0", "x1")]

                def dw1_tile(vt):
                    g1 = g1_sb[vt]
                    for ci, (s0, s1) in enumerate(dw1_chunks):
                        n = s1 - s0
                        cls = DW1_PAT[ci]
                        if cls == 'pe':
                            psP = psB.tile([128, CH], f32, tag="conv", bufs=2,
                                           name=f"psP{vt}_{s0}")
                            for tap in range(9):
                                dy, dx = tap // 3 - 1, tap % 3 - 1
                                off = PAD + s0 + dy * COLS + dx
                                nc.tensor.matmul(
                                    psP[:, :n], lhsT=dwd1_sb[vt][tap],
                                    rhs=v_sb[vt][:, off:off + n],
                                    start=(tap == 0), stop=(tap == 8))
                            nc.scalar.activation(g1[:, s0:s1],
                                                 psP[:, :n], ACT.Gelu)
                            continue
                        acc = scr()
                        for tap in range(9):
                            dy, dx = tap // 3 - 1, tap % 3 - 1
                            src = v_sb[vt][:, PAD + s0 + dy * COLS + dx:
                                           PAD + s0 + dy * COLS + dx + n]
                            wap = dwcT[9 + vt][:, tap:tap + 1]
                            if tap == 0:
                                if cls == 'dve':
                                    nc.vector.tensor_scalar_mul(
                                        acc[:, :n], src, wap)
                                else:
                                    nc.scalar.mul(acc[:, :n], src, wap)
                            else:
                                t2 = scr()
                                if cls == 'dve':
                                    nc.vector.tensor_scalar_mul(
                                        t2[:, :n], src, wap)
                                else:
                                    nc.scalar.mul(t2[:, :n], src, wap)
                                nc.vector.tensor_add(acc[:, :n], acc[:, :n],
                                                     t2[:, :n])
                        nc.scalar.activation(g1[:, s0:s1],
                                             acc[:, :n], ACT.Gelu)
                    g1v = g1.rearrange("p (r c) -> p r c", c=COLS)
                    nc.vector.memset(g1v[:, 2:68, 0:1], 0.0)
                    nc.vector.memset(g1v[:, 2:68, COLS - 1:COLS], 0.0)
                    nc.vector.tensor_scalar_mul(
                        g1v[:, 2], g1v[:, 2], ez_sb[:, 0:1])
                    nc.vector.tensor_scalar_mul(
                        g1v[:, 67], g1v[:, 67], ez_sb[:, 1:2])

                vviews = [v_sb[i][:, PAD:PAD + NPOS].rearrange(
                    "p (r c) -> p r c", c=COLS) for i in range(3)]

                def out_tile(ot):
                    g1view = g1_sb[ot].rearrange("p (r c) -> p r c", c=COLS)
                    for i in range(16):
                        psO = psB.tile([128, CH], f32, tag="dw", bufs=2,
                                       name=f"psO{ot}_{i}")
                        for tap in range(9):
                            dy, dx = tap // 3 - 1, tap % 3 - 1
                            rhs = g1view[:, 3 + 4 * i + dy:7 + 4 * i + dy,
                                         1 + dx:129 + dx]
                            nc.tensor.matmul(
                                psO, lhsT=dwd2_sb[ot][tap], rhs=rhs,
                                start=(tap == 0), stop=False)
                        for dtt in range(3):
                            rhs = vviews[dtt][:, 3 + 4 * i:7 + 4 * i, 1:129]
                            nc.tensor.matmul(
                                psO,
                                lhsT=MT_sb[dtt][:, 128 * ot:128 * ot + 128],
                                rhs=rhs, start=False, stop=(dtt == 2))
                        outf = pb.tile([128, CH], f32, tag="outf", bufs=2,
                                       name=f"outf{ot}_{i}")
                        nc.scalar.copy(outf, psO)
                        nc.sync.dma_start(
                            out_d.ap()[128 * ot:128 * ot + 128,
                                       CH * i:CH * i + CH], outf)

                # dw1(0) fills the AllReduce round-trip gap on every engine
                dw1_tile(0)

                # ============ softmax / attn / M build ============
                Gv = G_sb.rearrange("p (b c) -> p b c", b=HEADS)
                nrm = pb.tile([48, 16], f32, tag="nrm", name="nrm")
                inv = pb.tile([48, 16], f32, tag="inv", name="inv")
                # cols 2b = qnorm, 2b+1 = knorm (payload-block order)
                nc.scalar.sqrt(
                    nrm.rearrange("p (b u) -> p b u", b=HEADS),
                    Gv[:, :, 48:50])
                nc.vector.tensor_scalar_max(nrm, nrm, 1e-12)
                nc.vector.reciprocal(inv, nrm)

                for h in range(HEADS):
                    b = blk(h)
                    B = pb.tile([48, 48], f32, tag="B", bufs=2, name=f"B{h}")
                    nc.vector.tensor_scalar(
                        out=B, in0=Gv[:, b, 0:48],
                        scalar1=inv[:, 2 * b + 1:2 * b + 2],
                        scalar2=float(temp[h]),
                        op0=MM, op1=MM)
                    psb = psB.tile([48, 48], f32, tag="ptrE", bufs=2,
                                   name=f"psb{h}")
                    nc.tensor.transpose(psb, B, id48_sb)
                    A0 = pb.tile([48, 48], f32, tag="A0", bufs=2,
                                 name=f"A0{h}")
                    nc.vector.tensor_scalar_mul(
                        A0, psb, inv[:, 2 * b:2 * b + 1])
                    e0 = pb.tile([48, 48], f32, tag="e0", bufs=2,
                                 name=f"e0{h}")
                    s_ = pb.tile([48, 4], f32, tag="s", bufs=2, name=f"s{h}")
                    nc.scalar.activation(e0, A0, ACT.Exp,
                                         accum_out=s_[:, 0:1])
                    e1 = pb.tile([48, 48], f32, tag="e1", bufs=2,
                                 name=f"e1{h}")
                    nc.vector.tensor_mul(e1, e0,
                                         mm_sb[:, 48 * h:48 * h + 48])
                    nc.vector.tensor_reduce(
                        s_[:, 1:2], e1, axis=AX, op=ADD)
                    r_ = pb.tile([48, 4], f32, tag="r", bufs=2, name=f"r{h}")
                    nc.vector.reciprocal(r_[:, 0:2], s_[:, 0:2])
                    nc.vector.tensor_scalar_mul(r_[:, 0:1], r_[:, 0:1],
                                                float(wsm[0]))
                    nc.vector.tensor_scalar_mul(r_[:, 1:2], r_[:, 1:2],
                                                float(wsm[1]))
                    t0 = pb.tile([48, 48], f32, tag="t0", bufs=2,
                                 name=f"t0{h}")
                    nc.vector.tensor_scalar_mul(t0, e0, r_[:, 0:1])
                    af = pb.tile([48, 48], f32, tag="af", bufs=2,
                                 name=f"af{h}")
                    nc.vector.tensor_scalar_mul(af, e1, r_[:, 1:2])
                    nc.vector.tensor_add(af, af, t0)
                    nc.vector.tensor_copy(attn_bf[h], af)

                # M = proj @ blockdiag(attn), transposed into MT_sb
                M_sb = []
                for ot in range(3):
                    M_ps = psB.tile([128, DIM], f32, tag="mps", bufs=1,
                                    name=f"mps{ot}")
                    for h in range(HEADS):
                        nc.tensor.matmul(
                            M_ps[:, 48 * h:48 * h + 48],
                            lhsT=projT_sb[h][:, 128 * ot:128 * ot + 128],
                            rhs=attn_bf[h], start=True, stop=True)
                    t = pb.tile([128, DIM], bf16, tag=f"msb{ot}",
                                name=f"msb{ot}")
                    nc.vector.tensor_copy(t, M_ps)
                    M_sb.append(t)
                for ot in range(3):
                    for dtt in range(3):
                        psM = psB.tile([128, 128], bf16, tag="ptrE",
                                       bufs=2, name=f"psM{ot}_{dtt}")
                        nc.tensor.transpose(
                            psM, M_sb[ot][:, 128 * dtt:128 * dtt + 128],
                            ident_sb)
                        nc.vector.tensor_copy(
                            MT_sb[dtt][:, 128 * ot:128 * ot + 128], psM)

                # remaining pos tiles round-robined with the fused out
                # chunks: out(ot) on PE overlaps dw1(vt+1) on DVE/ACT
                dw1_tile(1)
                out_tile(0)
                dw1_tile(2)
                out_tile(1)
                out_tile(2)

    nc.compile()
    return nc


def _host_prep(x, mask, qkv_w, dw_w, proj_w, temperature, w_blend,
               pos_w1, pos_w2):
    x = np.asarray(x, np.float32)
    b = x.shape[0]
    xp = np.zeros((b, DIM, 134, COLS), np.float32)
    xp[:, :, 3:131, 1:129] = x
    shards = []
    for core in range(NCORES):
        bi, s = core // 2, core % 2
        shards.append(np.ascontiguousarray(
            xp[bi, :, 64 * s:64 * s + ROWS, :]).reshape(DIM, NPOS).astype(BF))
    wT = np.ascontiguousarray(
        np.asarray(qkv_w, np.float32)[:, :, 0, 0].T).astype(BF)
    dwd = np.zeros((15, 9, 128, 128), np.float32)
    dwk = np.asarray(dw_w, np.float32)[:, 0]       # (1152, 3, 3)
    pk1 = np.asarray(pos_w1, np.float32)[:, 0]     # (384, 3, 3)
    pk2 = np.asarray(pos_w2, np.float32)[:, 0]
    idx = np.arange(128)
    for t9 in range(9):
        for tap in range(9):
            dwd[t9, tap, idx, idx] = dwk[128 * t9:128 * t9 + 128,
                                         tap // 3, tap % 3]
    for vt in range(3):
        for tap in range(9):
            dwd[9 + vt, tap, idx, idx] = pk1[128 * vt:128 * vt + 128,
                                             tap // 3, tap % 3]
            dwd[12 + vt, tap, idx, idx] = pk2[128 * vt:128 * vt + 128,
                                              tap // 3, tap % 3]
    dwcol = np.zeros((15, 9, 128), np.float32)
    for t in range(15):
        for tap in range(9):
            dwcol[t, tap] = dwd[t, tap, idx, idx]
    dwd = dwd.astype(BF)
    pw = np.asarray(proj_w, np.float32)[:, :, 0, 0]
    projT = np.stack([np.ascontiguousarray(pw[:, 48 * h:48 * h + 48].T)
                      for h in range(HEADS)]).astype(BF)     # (8, 48, 384)
    mmul = (np.asarray(mask)[0] != 0).astype(np.float32)     # (8, 48, 48)
    maskmul = np.ascontiguousarray(
        mmul.transpose(1, 0, 2).reshape(48, HEADS * 48))
    ident = np.eye(128, dtype=np.float32).astype(BF)
    ident48 = np.eye(48, dtype=np.float32)
    wb = np.asarray(w_blend, np.float32)
    e = np.exp(wb - wb.max())
    wsm = e / e.sum()
    temp = np.asarray(temperature, np.float32).reshape(HEADS)
    return (shards, wT, dwd, dwcol, projT, maskmul, ident, ident48, wsm, temp)


def kernel(**inputs):
    from concourse import bass_utils
    (shards, wT, dwd, dwcol, projT, maskmul, ident, ident48, wsm,
     temp) = _host_prep(**inputs)
    key = (tuple(np.round(wsm, 8)), tuple(np.round(temp, 8)))
    if key not in _CACHE:
        _CACHE[key] = _build(wsm, temp)
    nc = _CACHE[key]
    in_maps = []
    for core in range(NCORES):
        s = core % 2
        ez = np.ones((128, 2), np.float32)
        ez[:, 0] = 0.0 if s == 0 else 1.0
        ez[:, 1] = 0.0 if s == 1 else 1.0
        in_maps.append({
            "x": shards[core], "wT": wT, "dwd": dwd, "dwcol": dwcol,
            "projT": projT, "maskmul": maskmul, "ident": ident,
            "ident48": ident48, "ez": ez,
        })
    res = bass_utils.run_bass_kernel_spmd(
        nc, in_maps, core_ids=list(range(NCORES)),
        trace=bool(int(__import__("os").environ.get("KBENCH_TRACE", "0"))))
    kernel._last_result = res
    x = np.asarray(inputs["x"])
    out = np.zeros((x.shape[0], DIM, 128, 128), np.float32)
    for core in range(NCORES):
        bi, s = core // 2, core % 2
        out[bi, :, 64 * s:64 * s + 64, :] = \
            np.asarray(res.results[core]["out"], np.float32).reshape(
                DIM, 64, 128)
    return out
